# revision 8
# baseline (speedup 1.0000x reference)
"""Fused TP-allreduce + bias/residual add + RMSNorm for Trainium2 (8 NeuronCores).

Strategy: the reference computes sum(x, axis=0) over the tp axis, then a
fused epilogue (bias + residual add, RMSNorm) on the [tokens, hidden] result.
Since this kernel receives the FULL inputs and distributes them itself, we
shard by TOKENS instead of tp-rank: core i gets x[:, i*1024:(i+1)*1024, :]
(all 8 tp slices for its token range) plus the matching residual rows and the
replicated bias/norm_weight. Each core reduces its 8 local slices and runs
the epilogue on its token shard — no inter-core communication at all. The
host concatenates the per-core output shards. This turns the problem into a
pure memory-bound streaming kernel (~176 MB HBM traffic per core).
"""

import numpy as np

TP = 8
TOKENS = 8192
HIDDEN = 4096
N_CORES = 8
TOK_PER_CORE = TOKENS // N_CORES  # 1024
P = 128  # SBUF partitions (token-tile height)
N_TILES = TOK_PER_CORE // P  # 8
EPS = 1e-6

_COMPILED = {}


def _broadcast_ap(ap, parts):
    """View a [N] DRAM AP as [parts, N] with partition stride 0."""
    import concourse.bass as bass

    return bass.AP(tensor=ap.tensor, offset=ap.offset, ap=[[0, parts]] + list(ap.ap))


def _build():
    import concourse.bacc as bacc
    import concourse.tile as tile
    from concourse import mybir

    f32 = mybir.dt.float32
    bf16 = mybir.dt.bfloat16
    nc = bacc.Bacc(
        "TRN2",
        target_bir_lowering=False,
        debug=False,
        enable_asserts=False,
        num_devices=N_CORES,
    )

    x = nc.dram_tensor("x", [TP, TOK_PER_CORE, HIDDEN], f32, kind="ExternalInput").ap()
    residual = nc.dram_tensor(
        "residual", [TOK_PER_CORE, HIDDEN], f32, kind="ExternalInput"
    ).ap()
    bias = nc.dram_tensor("bias", [HIDDEN], f32, kind="ExternalInput").ap()
    weight = nc.dram_tensor("norm_weight", [HIDDEN], f32, kind="ExternalInput").ap()
    norm_out = nc.dram_tensor(
        "norm_out", [TOK_PER_CORE, HIDDEN], f32, kind="ExternalOutput"
    ).ap()
    residual_out = nc.dram_tensor(
        "residual_out", [TOK_PER_CORE, HIDDEN], f32, kind="ExternalOutput"
    ).ap()

    with tile.TileContext(nc) as tc:
        with (
            tc.tile_pool(name="consts", bufs=1) as consts,
            tc.tile_pool(name="xp", bufs=10) as xp,
            tc.tile_pool(name="routp", bufs=2) as routp,
            tc.tile_pool(name="resp", bufs=2) as resp,
            tc.tile_pool(name="noutp", bufs=2) as noutp,
            tc.tile_pool(name="statp", bufs=4) as statp,
        ):
            # Load bias/norm_weight once (16 KB HBM reads), then replicate
            # across partitions with log-doubling SBUF->SBUF DMAs. A direct
            # partition-broadcast DMA from DRAM re-reads HBM once per
            # partition (4.2 MB of wasted HBM traffic).
            bias_t = consts.tile([P, HIDDEN], bf16)
            w_t = consts.tile([P, HIDDEN], bf16)
            for t, src in ((bias_t, bias), (w_t, weight)):
                nc.gpsimd.dma_start(out=t[0:1, :], in_=_broadcast_ap(src, 1))
                k = 1
                while k < P:
                    nc.sync.dma_start(out=t[k : 2 * k, :], in_=t[0:k, :])
                    k *= 2
            eps_t = consts.tile([P, 1], f32)
            nc.vector.memset(eps_t[:], EPS)

            n_groups = HIDDEN // nc.vector.BN_STATS_FMAX  # 8 subgroups of 512

            for it in range(N_TILES):
                t0 = it * P
                # Hidden-split the final tile: its loads/compute/stores
                # pipeline at half granularity, shortening the kernel tail
                # (everything after the last HBM read of x).
                n_chunks = 2 if it == N_TILES - 1 else 1
                cw = HIDDEN // n_chunks  # chunk width

                res_t = resp.tile([P, HIDDEN], f32)
                rout = routp.tile([P, HIDDEN], f32)
                nout = noutp.tile([P, HIDDEN], f32)
                stats = statp.tile([P, n_groups, nc.vector.BN_STATS_DIM], f32)

                for c in range(n_chunks):
                    h0 = c * cw
                    sl = slice(h0, h0 + cw)
                    nc.sync.dma_start(
                        out=res_t[:, sl], in_=residual[t0 : t0 + P, sl]
                    )
                    # rb = residual + bias, off the critical path (no x dep)
                    nc.vector.tensor_add(res_t[:, sl], res_t[:, sl], bias_t[:, sl])

                    # Cast-DMA (SWDGE) x slices f32->bf16: the tp-sum adds
                    # then run in the DVE 2x (16-bit) perf mode. Serial
                    # accumulate: after the LAST slice lands only one add
                    # remains on the critical path.
                    x_tiles = []
                    for i in range(TP):
                        xt = xp.tile([P, cw], bf16, tag="xtile")
                        nc.gpsimd.dma_start(out=xt[:], in_=x[i, t0 : t0 + P, sl])
                        x_tiles.append(xt)
                    for i in range(1, TP):
                        nc.vector.tensor_add(
                            x_tiles[0][:], x_tiles[0][:], x_tiles[i][:]
                        )
                    # residual_out = sum + (residual + bias)
                    nc.vector.tensor_add(rout[:, sl], x_tiles[0][:], res_t[:, sl])
                    nc.sync.dma_start(
                        out=residual_out[t0 : t0 + P, sl], in_=rout[:, sl]
                    )
                    # mean(x^2) = var + mean^2 via bn_stats/bn_aggr
                    gpc = cw // nc.vector.BN_STATS_FMAX
                    for g in range(gpc):
                        gg = c * gpc + g
                        nc.vector.bn_stats(
                            out=stats[:, gg, :],
                            in_=rout[:, gg * 512 : (gg + 1) * 512],
                        )

                mv = statp.tile([P, nc.vector.BN_AGGR_DIM], f32)
                nc.vector.bn_aggr(out=mv[:], in_=stats[:])
                # ms = mean^2 + var in one tensor_scalar
                ms = statp.tile([P, 1], f32)
                nc.vector.tensor_scalar(
                    out=ms[:],
                    in0=mv[:, 0:1],
                    scalar1=mv[:, 0:1],
                    scalar2=mv[:, 1:2],
                    op0=mybir.AluOpType.mult,
                    op1=mybir.AluOpType.add,
                )
                # rstd = 1/sqrt(ms + eps)
                rstd = statp.tile([P, 1], f32)
                nc.scalar.activation(
                    out=rstd[:],
                    in_=ms[:],
                    func=mybir.ActivationFunctionType.Sqrt,
                    bias=eps_t[:],
                )
                nc.vector.reciprocal(out=rstd[:], in_=rstd[:])

                # norm_out = residual_out * rstd * norm_weight
                # (rstd scale on the Scalar engine; weight mul on DVE)
                for c in range(n_chunks):
                    sl = slice(c * cw, (c + 1) * cw)
                    nc.scalar.activation(
                        out=nout[:, sl],
                        in_=rout[:, sl],
                        func=mybir.ActivationFunctionType.Copy,
                        scale=rstd[:],
                    )
                    nc.vector.tensor_mul(nout[:, sl], nout[:, sl], w_t[:, sl])
                    nc.scalar.dma_start(
                        out=norm_out[t0 : t0 + P, sl], in_=nout[:, sl]
                    )

    nc.compile()
    return nc


def _get_compiled():
    if "nc" not in _COMPILED:
        _COMPILED["nc"] = _build()
    return _COMPILED["nc"]


def _shard_inputs(x, bias, residual, norm_weight):
    x = np.ascontiguousarray(np.asarray(x, dtype=np.float32))
    bias = np.ascontiguousarray(np.asarray(bias, dtype=np.float32))
    residual = np.ascontiguousarray(np.asarray(residual, dtype=np.float32))
    norm_weight = np.ascontiguousarray(np.asarray(norm_weight, dtype=np.float32))
    in_maps = []
    for c in range(N_CORES):
        lo, hi = c * TOK_PER_CORE, (c + 1) * TOK_PER_CORE
        in_maps.append(
            {
                "x": np.ascontiguousarray(x[:, lo:hi, :]),
                "residual": residual[lo:hi],
                "bias": bias,
                "norm_weight": norm_weight,
            }
        )
    return in_maps


def run(inputs, trace=False):
    """Run the SPMD kernel. Returns ((norm_out, residual_out), BassKernelResults)."""
    from concourse.bass_utils import run_bass_kernel_spmd

    nc = _get_compiled()
    in_maps = _shard_inputs(
        inputs["x"], inputs["bias"], inputs["residual"], inputs["norm_weight"]
    )
    res = run_bass_kernel_spmd(nc, in_maps, core_ids=list(range(N_CORES)), trace=trace)
    norm = np.concatenate([res.results[c]["norm_out"] for c in range(N_CORES)], axis=0)
    rout = np.concatenate(
        [res.results[c]["residual_out"] for c in range(N_CORES)], axis=0
    )
    return (norm, rout), res


def kernel(x, bias, residual, norm_weight, **_unused):
    (norm, rout), _ = run(
        {"x": x, "bias": bias, "residual": residual, "norm_weight": norm_weight}
    )
    return norm, rout


# revision 9
# speedup vs baseline: 1.1241x; 1.1241x over previous
"""Fused TP-allreduce + bias/residual add + RMSNorm for Trainium2 (8 NeuronCores).

Strategy: the reference computes sum(x, axis=0) over the tp axis, then a
fused epilogue (bias + residual add, RMSNorm) on the [tokens, hidden] result.
Since this kernel receives the FULL inputs and distributes them itself, we
shard by TOKENS instead of tp-rank: core i gets x[:, i*1024:(i+1)*1024, :]
(all 8 tp slices for its token range) plus the matching residual rows and the
replicated bias/norm_weight. Each core reduces its 8 local slices and runs
the epilogue on its token shard — no inter-core communication at all. The
host concatenates the per-core output shards. This turns the problem into a
pure memory-bound streaming kernel (~176 MB HBM traffic per core).
"""

import numpy as np

TP = 8
TOKENS = 8192
HIDDEN = 4096
N_CORES = 8
TOK_PER_CORE = TOKENS // N_CORES  # 1024
P = 128  # SBUF partitions (token-tile height)
N_TILES = TOK_PER_CORE // P  # 8
EPS = 1e-6

_COMPILED = {}


def _broadcast_ap(ap, parts):
    """View a [N] DRAM AP as [parts, N] with partition stride 0."""
    import concourse.bass as bass

    return bass.AP(tensor=ap.tensor, offset=ap.offset, ap=[[0, parts]] + list(ap.ap))


def _build():
    import concourse.bacc as bacc
    import concourse.tile as tile
    from concourse import mybir

    f32 = mybir.dt.float32
    bf16 = mybir.dt.bfloat16
    nc = bacc.Bacc(
        "TRN2",
        target_bir_lowering=False,
        debug=False,
        enable_asserts=False,
        num_devices=N_CORES,
    )

    x = nc.dram_tensor("x", [TP, TOK_PER_CORE, HIDDEN], f32, kind="ExternalInput").ap()
    residual = nc.dram_tensor(
        "residual", [TOK_PER_CORE, HIDDEN], f32, kind="ExternalInput"
    ).ap()
    bias = nc.dram_tensor("bias", [HIDDEN], f32, kind="ExternalInput").ap()
    weight = nc.dram_tensor("norm_weight", [HIDDEN], f32, kind="ExternalInput").ap()
    norm_out = nc.dram_tensor(
        "norm_out", [TOK_PER_CORE, HIDDEN], f32, kind="ExternalOutput"
    ).ap()
    residual_out = nc.dram_tensor(
        "residual_out", [TOK_PER_CORE, HIDDEN], f32, kind="ExternalOutput"
    ).ap()

    with tile.TileContext(nc) as tc:
        with (
            tc.tile_pool(name="consts", bufs=1) as consts,
            tc.tile_pool(name="xp", bufs=6) as xp,
            tc.tile_pool(name="routp", bufs=2) as routp,
            tc.tile_pool(name="resp", bufs=2) as resp,
            tc.tile_pool(name="noutp", bufs=2) as noutp,
            tc.tile_pool(name="statp", bufs=4) as statp,
        ):
            # Load bias/norm_weight once (16 KB HBM reads), then replicate
            # across partitions with log-doubling SBUF->SBUF DMAs. A direct
            # partition-broadcast DMA from DRAM re-reads HBM once per
            # partition (4.2 MB of wasted HBM traffic).
            bias_t = consts.tile([P, HIDDEN], bf16)
            w_t = consts.tile([P, HIDDEN], bf16)
            for t, src in ((bias_t, bias), (w_t, weight)):
                nc.gpsimd.dma_start(out=t[0:1, :], in_=_broadcast_ap(src, 1))
                k = 1
                while k < P:
                    nc.sync.dma_start(out=t[k : 2 * k, :], in_=t[0:k, :])
                    k *= 2
            eps_t = consts.tile([P, 1], f32)
            nc.vector.memset(eps_t[:], EPS)

            n_groups = HIDDEN // nc.vector.BN_STATS_FMAX  # 8 subgroups of 512

            for it in range(N_TILES):
                t0 = it * P
                # Hidden-split the final tile: its loads/compute/stores
                # pipeline at half granularity, shortening the kernel tail
                # (everything after the last HBM read of x).
                n_chunks = 2 if it == N_TILES - 1 else 1
                cw = HIDDEN // n_chunks  # chunk width

                res_t = resp.tile([P, HIDDEN], f32)
                rout = routp.tile([P, HIDDEN], f32)
                nout = noutp.tile([P, HIDDEN], f32)
                stats = statp.tile([P, n_groups, nc.vector.BN_STATS_DIM], f32)

                for c in range(n_chunks):
                    h0 = c * cw
                    sl = slice(h0, h0 + cw)
                    nc.sync.dma_start(
                        out=res_t[:, sl], in_=residual[t0 : t0 + P, sl]
                    )
                    # rb = residual + bias, off the critical path (no x dep)
                    nc.vector.tensor_add(res_t[:, sl], res_t[:, sl], bias_t[:, sl])

                    # Cast-DMA (SWDGE) x slices f32->bf16: the tp-sum adds
                    # then run in the DVE 2x (16-bit) perf mode. Serial
                    # accumulate: after the LAST slice lands only one add
                    # remains on the critical path.
                    x_tiles = []
                    for i in range(TP):
                        xt = xp.tile([P, cw], bf16, tag="xtile")
                        nc.gpsimd.dma_start(out=xt[:], in_=x[i, t0 : t0 + P, sl])
                        x_tiles.append(xt)
                    for i in range(1, TP):
                        nc.vector.tensor_add(
                            x_tiles[0][:], x_tiles[0][:], x_tiles[i][:]
                        )
                    # residual_out = sum + (residual + bias)
                    nc.vector.tensor_add(rout[:, sl], x_tiles[0][:], res_t[:, sl])
                    nc.sync.dma_start(
                        out=residual_out[t0 : t0 + P, sl], in_=rout[:, sl]
                    )
                    # mean(x^2) = var + mean^2 via bn_stats/bn_aggr
                    gpc = cw // nc.vector.BN_STATS_FMAX
                    for g in range(gpc):
                        gg = c * gpc + g
                        nc.vector.bn_stats(
                            out=stats[:, gg, :],
                            in_=rout[:, gg * 512 : (gg + 1) * 512],
                        )

                mv = statp.tile([P, nc.vector.BN_AGGR_DIM], f32)
                nc.vector.bn_aggr(out=mv[:], in_=stats[:])
                # ms = mean^2 + var in one tensor_scalar
                ms = statp.tile([P, 1], f32)
                nc.vector.tensor_scalar(
                    out=ms[:],
                    in0=mv[:, 0:1],
                    scalar1=mv[:, 0:1],
                    scalar2=mv[:, 1:2],
                    op0=mybir.AluOpType.mult,
                    op1=mybir.AluOpType.add,
                )
                # rstd = 1/sqrt(ms + eps)
                rstd = statp.tile([P, 1], f32)
                nc.scalar.activation(
                    out=rstd[:],
                    in_=ms[:],
                    func=mybir.ActivationFunctionType.Sqrt,
                    bias=eps_t[:],
                )
                nc.vector.reciprocal(out=rstd[:], in_=rstd[:])

                # norm_out = residual_out * rstd * norm_weight
                # (rstd scale on the Scalar engine; weight mul on DVE)
                for c in range(n_chunks):
                    sl = slice(c * cw, (c + 1) * cw)
                    nc.scalar.activation(
                        out=nout[:, sl],
                        in_=rout[:, sl],
                        func=mybir.ActivationFunctionType.Copy,
                        scale=rstd[:],
                    )
                    nc.vector.tensor_mul(nout[:, sl], nout[:, sl], w_t[:, sl])
                    nc.scalar.dma_start(
                        out=norm_out[t0 : t0 + P, sl], in_=nout[:, sl]
                    )

    nc.compile()
    return nc


def _get_compiled():
    if "nc" not in _COMPILED:
        _COMPILED["nc"] = _build()
    return _COMPILED["nc"]


def _shard_inputs(x, bias, residual, norm_weight):
    x = np.ascontiguousarray(np.asarray(x, dtype=np.float32))
    bias = np.ascontiguousarray(np.asarray(bias, dtype=np.float32))
    residual = np.ascontiguousarray(np.asarray(residual, dtype=np.float32))
    norm_weight = np.ascontiguousarray(np.asarray(norm_weight, dtype=np.float32))
    in_maps = []
    for c in range(N_CORES):
        lo, hi = c * TOK_PER_CORE, (c + 1) * TOK_PER_CORE
        in_maps.append(
            {
                "x": np.ascontiguousarray(x[:, lo:hi, :]),
                "residual": residual[lo:hi],
                "bias": bias,
                "norm_weight": norm_weight,
            }
        )
    return in_maps


def run(inputs, trace=False):
    """Run the SPMD kernel. Returns ((norm_out, residual_out), BassKernelResults)."""
    from concourse.bass_utils import run_bass_kernel_spmd

    nc = _get_compiled()
    in_maps = _shard_inputs(
        inputs["x"], inputs["bias"], inputs["residual"], inputs["norm_weight"]
    )
    res = run_bass_kernel_spmd(nc, in_maps, core_ids=list(range(N_CORES)), trace=trace)
    norm = np.concatenate([res.results[c]["norm_out"] for c in range(N_CORES)], axis=0)
    rout = np.concatenate(
        [res.results[c]["residual_out"] for c in range(N_CORES)], axis=0
    )
    return (norm, rout), res


def kernel(x, bias, residual, norm_weight, **_unused):
    (norm, rout), _ = run(
        {"x": x, "bias": bias, "residual": residual, "norm_weight": norm_weight}
    )
    return norm, rout
